# revision 1
# baseline (speedup 1.0000x reference)
"""CGConvNet (gnn_message_passing) Trainium2 kernel.

Strategy (8 NeuronCores, SPMD single program):
  - Nodes padded 50000 -> 50176 and sharded into 8 contiguous ranges of 6272
    (49 blocks of 128). Edges are assigned to the core owning their dst node.
  - Per core, edges are split into group A (src < 32768) and group B
    (src >= 32768) so gather indices fit in int16, then grouped by the
    128-node destination block with a uniform tiles-per-block padding so the
    program structure is identical on every core.
  - Per conv layer: each core computes its shard of the src-side table
    (v @ w_src) which is AllGathered into a full [50176, 64] DRAM table, and
    its local dst-side table (v @ w_dst).  Edge messages are assembled with
    dma_gather (256B rows), elementwise gate on ACT/DVE, and scatter-added
    into node blocks with one-hot matmuls accumulated in PSUM.
  - Graph pooling: one-hot matmul into a [64, 512] transposed graph frame,
    AllReduce across cores, then the tiny lin2 head.
"""

import math
import os
import sys

import numpy as np

sys.path.insert(0, "/opt/trn_rl_repo")

import concourse.bacc as bacc  # noqa: E402
import concourse.bass as bass  # noqa: E402
import concourse.mybir as mybir  # noqa: E402
import concourse.tile as tile  # noqa: E402
from concourse.bass_utils import run_bass_kernel_spmd  # noqa: E402
from concourse.library_config import mlp  # noqa: E402
from concourse.masks import make_identity  # noqa: E402

F32 = mybir.dt.float32
I16 = mybir.dt.int16
I32 = mybir.dt.int32
AF = mybir.ActivationFunctionType


class CFG:
    def __init__(self, n_nodes=50000, n_edges=800000, atom=100, bond=41,
                 hid=64, n_conv=3, num_graphs=500, n_cores=8, graph_frame=512):
        self.n_nodes, self.n_edges = n_nodes, n_edges
        self.atom, self.bond, self.hid = atom, bond, hid
        self.n_conv, self.num_graphs, self.n_cores = n_conv, num_graphs, n_cores
        # padded node count: multiple of 128 * n_cores
        q = 128 * n_cores
        self.npad = ((n_nodes + q - 1) // q) * q
        self.sh = self.npad // n_cores          # nodes per core
        self.nb = self.sh // 128                # 128-node blocks per core
        self.split = min(32768, (self.npad // 2 // 128) * 128)
        self.graph_frame = graph_frame
        assert num_graphs <= graph_frame
        assert self.npad - self.split <= 32768


REAL = CFG()


def _wrap_idx(a):
    """int16 index array -> [128, n/16] wrapped+replicated layout."""
    assert a.size % 16 == 0
    w = a.reshape(-1, 16).T  # [16, n/16]
    return np.tile(w, (8, 1)).copy()


def _prep(cfg, inputs):
    """Host-side data prep. Returns (in_maps, meta)."""
    src = np.asarray(inputs["src"]).astype(np.int64)
    dst = np.asarray(inputs["dst"]).astype(np.int64)
    gdf = np.asarray(inputs["gdf_feat"]).astype(np.float32)
    n_feat = np.asarray(inputs["n_feat"]).astype(np.float32)
    graph_ids = np.asarray(inputs["graph_ids"]).astype(np.int64)

    C, SH, NB, SPLIT = cfg.n_cores, cfg.sh, cfg.nb, cfg.split
    core_of = dst // SH

    # pass 1: per (core, group, block) counts -> uniform tile counts
    Tg = [0, 0]
    per = {}
    for c in range(C):
        in_core = core_of == c
        for g in range(2):
            m = in_core & ((src < SPLIT) if g == 0 else (src >= SPLIT))
            e = np.nonzero(m)[0]
            s = src[e]
            dloc = dst[e] - c * SH
            blk = dloc >> 7
            order = np.argsort(blk, kind="stable")
            e, s, dloc, blk = e[order], s[order], dloc[order], blk[order]
            counts = np.bincount(blk, minlength=NB)
            Tg[g] = max(Tg[g], int(math.ceil(counts.max() / 128.0)))
            per[(c, g)] = (e, s, dloc, blk, counts)
    TA, TB = max(Tg[0], 1), max(Tg[1], 1)

    # weights
    W0b = np.vstack([np.asarray(inputs["lin0_w"], np.float32),
                     np.asarray(inputs["lin0_b"], np.float32)[None]])
    conv_w = np.asarray(inputs["conv_w"], np.float32)
    H = cfg.hid
    Ws = np.ascontiguousarray(conv_w[:H])
    Wd = np.ascontiguousarray(conv_w[H:2 * H])
    Web = np.vstack([conv_w[2 * H:],
                     np.asarray(inputs["conv_b"], np.float32)[None]])
    w2 = np.asarray(inputs["lin2_w"], np.float32).reshape(H, 1)
    b2f = float(np.asarray(inputs["lin2_b"], np.float32).reshape(-1)[0])

    nf_pad = np.zeros((cfg.npad, cfg.atom), np.float32)
    nf_pad[:cfg.n_nodes] = n_feat
    gid_pad = np.full(cfg.npad, -1.0, np.float32)
    gid_pad[:cfg.n_nodes] = graph_ids.astype(np.float32)

    in_maps = []
    for c in range(C):
        m = {}
        # pass 2: padded per-group edge arrays
        for g, T in ((0, TA), (1, TB)):
            E = NB * T * 128
            e, s, dloc, blk, counts = per[(c, g)]
            starts = np.concatenate([[0], np.cumsum(counts)])
            within = np.arange(len(s)) - np.repeat(starts[:-1], counts)
            pos = blk * (T * 128) + within
            srcidx = np.zeros(E, np.int16)
            srcidx[pos] = (s if g == 0 else s - SPLIT).astype(np.int16)
            dstidx = np.zeros(E, np.int16)
            dstidx[pos] = dloc.astype(np.int16)
            dstrel = np.full(E, -1.0, np.float32)
            dstrel[pos] = (dloc - (blk << 7)).astype(np.float32)
            gdfT = np.zeros((cfg.bond + 1, E), np.float32)
            gdfT[:cfg.bond, pos] = gdf[e].T
            gdfT[cfg.bond, pos] = 1.0
            tag = "AB"[g]
            m[f"si{tag}"] = _wrap_idx(srcidx)
            m[f"di{tag}"] = _wrap_idx(dstidx)
            m[f"dr{tag}"] = np.ascontiguousarray(dstrel.reshape(-1, 128).T)
            m[f"gdfT{tag}"] = gdfT
        nfT = np.zeros((cfg.atom + 1, SH), np.float32)
        nfT[:cfg.atom] = nf_pad[c * SH:(c + 1) * SH].T
        nfT[cfg.atom] = 1.0
        m["nfT"] = nfT
        m["grel"] = np.ascontiguousarray(
            gid_pad[c * SH:(c + 1) * SH].reshape(NB, 128).T)
        m.update(W0b=W0b, Ws=Ws, Wd=Wd, Web=Web, w2=w2)
        in_maps.append(m)

    meta = dict(TA=TA, TB=TB, b2f=b2f)
    return in_maps, meta


def _build(cfg, meta, sim_safe_gate=True, debug_level=99):
    """Build the Bass program. Returns compiled nc."""
    C, SH, NB, SPLIT, H = cfg.n_cores, cfg.sh, cfg.nb, cfg.split, cfg.hid
    NPAD, GF = cfg.npad, cfg.graph_frame
    TA, TB, b2f = meta["TA"], meta["TB"], meta["b2f"]
    EA, EB = NB * TA * 128, NB * TB * 128
    AT1, BD1 = cfg.atom + 1, cfg.bond + 1
    CB = int(os.environ.get('CGK_CB', '2'))  # blocks per edge-chunk
    NQ = int(os.environ.get('CGK_NQ', '1'))

    def QN(q):
        return q % NQ

    nc = bacc.Bacc("TRN2", target_bir_lowering=False, debug=False,
                   num_devices=C, num_swdge_queues=4)

    din = {}
    for tag, T, E in (("A", TA, EA), ("B", TB, EB)):
        din[f"si{tag}"] = nc.dram_tensor(f"si{tag}", [128, E // 16], I16,
                                         kind="ExternalInput")
        din[f"di{tag}"] = nc.dram_tensor(f"di{tag}", [128, E // 16], I16,
                                         kind="ExternalInput")
        din[f"dr{tag}"] = nc.dram_tensor(f"dr{tag}", [128, E // 128], F32,
                                         kind="ExternalInput")
        din[f"gdfT{tag}"] = nc.dram_tensor(f"gdfT{tag}", [BD1, E], F32,
                                           kind="ExternalInput")
    din["nfT"] = nc.dram_tensor("nfT", [AT1, SH], F32, kind="ExternalInput")
    din["grel"] = nc.dram_tensor("grel", [128, NB], F32, kind="ExternalInput")
    din["W0b"] = nc.dram_tensor("W0b", [AT1, H], F32, kind="ExternalInput")
    din["Ws"] = nc.dram_tensor("Ws", [H, H], F32, kind="ExternalInput")
    din["Wd"] = nc.dram_tensor("Wd", [H, H], F32, kind="ExternalInput")
    din["Web"] = nc.dram_tensor("Web", [BD1, H], F32, kind="ExternalInput")
    din["w2"] = nc.dram_tensor("w2", [H, 1], F32, kind="ExternalInput")
    preds_out = nc.dram_tensor("preds", [1, GF], F32, kind="ExternalOutput")

    # chunk plan over blocks
    chunks = []
    b = 0
    while b < NB:
        n = min(CB, NB - b)
        chunks.append((b, n))
        b += n

    with tile.TileContext(nc) as tc:
        with tc.tile_pool(name="const", bufs=1) as constp, \
             tc.tile_pool(name="slab", bufs=1) as slabp, \
             tc.tile_pool(name="ph0", bufs=2) as ph0, \
             tc.tile_pool(name="work", bufs=2) as work, \
             tc.tile_pool(name="work3", bufs=3) as work3, \
             tc.tile_pool(name="small", bufs=2) as small, \
             tc.tile_pool(name="repp", bufs=1) as repp, \
             tc.tile_pool(name="psum", bufs=2, space="PSUM") as psum, \
             tc.tile_pool(name="psumt", bufs=1, space="PSUM") as psumt, \
             tc.tile_pool(name="psacc", bufs=2, space="PSUM") as psacc, \
             tc.tile_pool(name="psg", bufs=1, space="PSUM") as psgp, \
             tc.tile_pool(name="dram", bufs=1, space="DRAM") as dram:

            nc.gpsimd.load_library(mlp)

            # ---- constants ----
            w0b = constp.tile([AT1, H], F32)
            nc.sync.dma_start(w0b[:], din["W0b"][:])
            ws = constp.tile([H, H], F32)
            nc.sync.dma_start(ws[:], din["Ws"][:])
            wd = constp.tile([H, H], F32)
            nc.sync.dma_start(wd[:], din["Wd"][:])
            web = constp.tile([BD1, H], F32)
            nc.sync.dma_start(web[:], din["Web"][:])
            w2 = constp.tile([H, 1], F32)
            nc.sync.dma_start(w2[:], din["w2"][:])
            ident = constp.tile([128, 128], F32)
            make_identity(nc, ident[:])
            iota_i = constp.tile([128, 128], I32)
            nc.gpsimd.iota(iota_i[:], pattern=[[1, 128]], base=0,
                           channel_multiplier=0)
            iota_f = constp.tile([128, 128], F32)
            nc.vector.tensor_copy(iota_f[:], iota_i[:])
            iotg_i = constp.tile([128, GF], I32)
            nc.gpsimd.iota(iotg_i[:], pattern=[[1, GF]], base=0,
                           channel_multiplier=0)
            iotg_f = constp.tile([128, GF], F32)
            nc.vector.tensor_copy(iotg_f[:], iotg_i[:])
            grel = constp.tile([128, NB], F32)
            nc.sync.dma_start(grel[:], din["grel"][:])

            # index / dst_rel slabs (reused every layer)
            idx_sb = {}
            for tag, E in (("A", EA), ("B", EB)):
                for kind in ("si", "di"):
                    t = slabp.tile([128, E // 16], I16, tag=f"slab_{kind}{tag}")
                    nc.sync.dma_start(t[:], din[f"{kind}{tag}"][:])
                    idx_sb[f"{kind}{tag}"] = t
                t = slabp.tile([128, E // 128], F32, tag=f"slab_dr{tag}")
                nc.sync.dma_start(t[:], din[f"dr{tag}"][:])
                idx_sb[f"dr{tag}"] = t

            # persistent slabs
            accT = slabp.tile([H, SH], F32)          # transposed node feats
            acc3 = slabp.tile([128, NB, H], F32)     # final-layer node feats

            # internal DRAM
            egdf_dA = dram.tile([128, NB * TA, H], F32)
            egdf_dB = dram.tile([128, NB * TB, H], F32)
            egdf_d = {"A": egdf_dA, "B": egdf_dB}
            ts_full = dram.tile([NPAD, H], F32)      # AllGathered src table
            agS_in = dram.tile([SH, H], F32)
            vwD_d = dram.tile([SH, H], F32)
            rep_in = dram.tile([H, GF], F32)
            rep_out = dram.tile([H, GF], F32)

            # ---- phase 0a: egdf = gdfT.T @ Web (per group, once) ----
            for tag, T in (("A", TA), ("B", TB)):
                ntile_tot = NB * T
                step = 6
                for t0 in range(0, ntile_tot, step):
                    nt = min(step, ntile_tot - t0)
                    gsb = ph0.tile([BD1, nt * 128], F32, tag="gdfchunk")
                    nc.sync.dma_start(
                        gsb[:], din[f"gdfT{tag}"][:, t0 * 128:(t0 + nt) * 128])
                    esb = ph0.tile([128, nt, H], F32, tag="egdfchunk")
                    for t in range(nt):
                        ps = psum.tile([128, H], F32, tag="mm64")
                        nc.tensor.matmul(ps[:],
                                         lhsT=gsb[:, t * 128:(t + 1) * 128],
                                         rhs=web[:], start=True, stop=True)
                        nc.scalar.copy(esb[:, t, :], ps[:])
                    nc.sync.dma_start(
                        egdf_d[tag][:, t0:t0 + nt, :], esb[:])

            # ---- phase 0b: lin0 -> accT ----
            for blk in range(NB):
                nf_t = small.tile([AT1, 128], F32, tag="nf")
                nc.sync.dma_start(
                    nf_t[:], din["nfT"][:, blk * 128:(blk + 1) * 128])
                ps = psum.tile([128, H], F32, tag="mm64")
                nc.tensor.matmul(ps[:], lhsT=nf_t[:], rhs=w0b[:],
                                 start=True, stop=True)
                v0 = small.tile([128, H], F32, tag="v0")
                nc.scalar.activation(v0[:], ps[:], AF.Relu)
                pst = psumt.tile([H, 128], F32, tag="mmT")
                nc.tensor.transpose(pst[:], v0[:], ident[:])
                nc.scalar.copy(accT[:, blk * 128:(blk + 1) * 128], pst[:])

            _unused = None

            def _dbg_out(src_ap):
                pr = repp.tile([1, GF], F32, tag="reprelu")
                nc.scalar.copy(pr[:], src_ap)
                nc.sync.dma_start(preds_out[:], pr[:])

            if debug_level == 1:
                _dbg_out(accT[0:1, 0:GF])
            # ---- conv layers ----
            n_layers = 0 if debug_level == 1 else cfg.n_conv
            if debug_level == 2:
                n_layers = 1
            for layer in range(n_layers):
                last = layer == n_layers - 1
                # tables
                for blk in range(NB):
                    col = slice(blk * 128, (blk + 1) * 128)
                    ps = psum.tile([128, H], F32, tag="mm64")
                    nc.tensor.matmul(ps[:], lhsT=accT[:, col], rhs=ws[:],
                                     start=True, stop=True)
                    t_s = small.tile([128, H], F32, tag="tbl")
                    nc.scalar.copy(t_s[:], ps[:])
                    nc.sync.dma_start(agS_in[col, :], t_s[:])
                    ps2 = psum.tile([128, H], F32, tag="mm64")
                    nc.tensor.matmul(ps2[:], lhsT=accT[:, col], rhs=wd[:],
                                     start=True, stop=True)
                    t_d = small.tile([128, H], F32, tag="tbl")
                    nc.scalar.copy(t_d[:], ps2[:])
                    nc.sync.dma_start(vwD_d[col, :], t_d[:])
                if os.environ.get("CGK_NO_AG", "0") == "1":
                    nc.sync.dma_start(ts_full[:][0:SH, :], agS_in[:])
                else:
                    nc.gpsimd.collective_compute(
                        "AllGather", mybir.AluOpType.bypass,
                        replica_groups=[list(range(C))],
                        ins=[agS_in.opt()], outs=[ts_full.opt()])

                ts_lo = ts_full[:][0:SPLIT, :]
                ts_hi = ts_full[:][SPLIT:NPAD, :]

                sub = int(os.environ.get("CGK_SUB", "9"))
                for ci, (b0, nblk) in enumerate(chunks if sub >= 1 else []):
                    bufs = {}
                    for gi, (tag, T) in enumerate((("A", TA), ("B", TB))):
                        nt = nblk * T
                        t_lo = b0 * T
                        e_lo = t_lo * 128
                        # stable queue per gather stream: srcA=0 dstA=1
                        # srcB=2 dstB=3 (rotating queues trips SWDGE sem-lane
                        # locking in the scheduler model)
                        GMAX = int(os.environ.get("CGK_GMAX", "1024"))
                        zp = work3 if tag == "A" else work
                        z = zp.tile([128, nt, H], F32, tag=f"z{tag}")
                        for s0 in range(0, nt * 128, GMAX):
                            ni = min(GMAX, nt * 128 - s0)
                            nc.gpsimd.dma_gather(
                                z[:, s0 // 128:(s0 + ni) // 128, :],
                                ts_lo if tag == "A" else ts_hi,
                                idx_sb[f"si{tag}"][:, (e_lo + s0) // 16:(e_lo + s0 + ni) // 16],
                                ni, ni, H, queue_num=QN(2 * gi))
                        gd = work.tile([128, nt, H], F32, tag=f"gd{tag}")
                        for s0 in range(0, nt * 128, GMAX):
                            ni = min(GMAX, nt * 128 - s0)
                            nc.gpsimd.dma_gather(
                                gd[:, s0 // 128:(s0 + ni) // 128, :],
                                vwD_d[:],
                                idx_sb[f"di{tag}"][:, (e_lo + s0) // 16:(e_lo + s0 + ni) // 16],
                                ni, ni, H, queue_num=QN(2 * gi + 1))
                        eg = work.tile([128, nt, H], F32, tag=f"eg{tag}")
                        if os.environ.get("CGK_NO_EGL", "0") == "1":
                            nc.gpsimd.memset(eg[:], 0.1)
                        else:
                            nc.sync.dma_start(eg[:],
                                              egdf_d[tag][:, t_lo:t_lo + nt, :])
                        if sub < 3:
                            bufs[tag] = (z, None, T)
                            continue
                        # z = gS + gD + egdf
                        nc.vector.tensor_add(z[:], z[:], gd[:])
                        nc.vector.tensor_add(z[:], z[:], eg[:])
                        # gate: msg = sigmoid(z) * leaky_relu(z, 0.01)
                        sg = work.tile([128, nt, H], F32, tag=f"sg{tag}")
                        nc.scalar.activation(sg[:], z[:], AF.Sigmoid)
                        if sim_safe_gate:
                            # leaky = 0.505*z + 0.495*|z| (gd, eg reused)
                            nc.scalar.activation(gd[:], z[:], AF.Abs,
                                                 scale=0.495)
                            nc.scalar.mul(eg[:], z[:], 0.505)
                            lk = gd
                            nc.vector.tensor_add(lk[:], gd[:], eg[:])
                        else:
                            lk = gd
                            nc.scalar.activation(lk[:], z[:], AF.Lrelu,
                                                 alpha=0.01)
                        msg = z
                        nc.vector.tensor_mul(msg[:], sg[:], lk[:])
                        if sub < 4:
                            bufs[tag] = (msg, None, T)
                            continue
                        # one-hot vs dst_rel
                        oh = work.tile([128, nt, 128], F32, tag=f"oh{tag}")
                        dr = idx_sb[f"dr{tag}"][:, t_lo:t_lo + nt]
                        dr_b = bass.AP(dr.tensor, dr.offset,
                                       [dr.ap[0], dr.ap[1], [0, 128]])
                        io = iota_f[:]
                        io_b = bass.AP(io.tensor, io.offset,
                                       [io.ap[0], [0, nt], io.ap[1]])
                        nc.vector.tensor_tensor(out=oh[:], in0=dr_b, in1=io_b,
                                                op=mybir.AluOpType.is_equal)
                        bufs[tag] = (msg, oh, T)

                    for bi in range(nblk if sub >= 5 else 0):
                        blk = b0 + bi
                        seq = []
                        for tag in ("A", "B"):
                            msg, oh, T = bufs[tag]
                            for t in range(T):
                                seq.append((msg[:, bi * T + t, :],
                                            oh[:, bi * T + t, :]))
                        if not last:
                            ps = psacc.tile([H, 128], F32, tag="accps")
                            for k, (m_ap, o_ap) in enumerate(seq):
                                nc.tensor.matmul(ps[:], lhsT=m_ap, rhs=o_ap,
                                                 start=(k == 0),
                                                 stop=(k == len(seq) - 1))
                            nc.scalar.copy(
                                accT[:, blk * 128:(blk + 1) * 128], ps[:])
                        else:
                            ps = psacc.tile([128, H], F32, tag="accps")
                            for k, (m_ap, o_ap) in enumerate(seq):
                                nc.tensor.matmul(ps[:], lhsT=o_ap, rhs=m_ap,
                                                 start=(k == 0),
                                                 stop=(k == len(seq) - 1))
                            nc.scalar.copy(acc3[:, blk, :], ps[:])

            # ---- graph pooling ----
            if debug_level == 2:
                if int(os.environ.get("CGK_SUB", "9")) >= 5:
                    _dbg_out(acc3[0:1, :, :].rearrange("a b c -> a (b c)")[:, 0:GF])
                else:
                    _dbg_out(accT[0:1, 0:GF])
            do_pool = debug_level >= 3
            psg = None
            if do_pool:
                psg = psgp.tile([H, GF], F32, tag="repps")
            for blk in range(NB if do_pool else 0):
                ohg = small.tile([128, GF], F32, tag="ohg")
                gcol = grel[:, blk:blk + 1]
                nc.vector.tensor_tensor(
                    out=ohg[:], in0=gcol.to_broadcast([128, GF]),
                    in1=iotg_f[:], op=mybir.AluOpType.is_equal)
                nc.tensor.matmul(psg[:], lhsT=acc3[:, blk, :], rhs=ohg[:],
                                 start=(blk == 0), stop=(blk == NB - 1))
            if do_pool:
                rep_s = repp.tile([H, GF], F32, tag="reps")
                nc.scalar.copy(rep_s[:], psg[:])
                nc.sync.dma_start(rep_in[:], rep_s[:])
                if debug_level >= 4:
                    nc.gpsimd.collective_compute(
                        "AllReduce", mybir.AluOpType.add,
                        replica_groups=[list(range(C))],
                        ins=[rep_in.opt()], outs=[rep_out.opt()])
                else:
                    nc.sync.dma_start(rep_out[:], rep_s[:])
                rep_r = repp.tile([H, GF], F32, tag="reps")
                nc.sync.dma_start(rep_r[:], rep_out[:])
                rep_relu = repp.tile([H, GF], F32, tag="reprelu")
                nc.scalar.activation(rep_relu[:], rep_r[:], AF.Relu)
                psp = psgp.tile([1, GF], F32, tag="repps")
                nc.tensor.matmul(psp[:], lhsT=w2[:], rhs=rep_relu[:],
                                 start=True, stop=True)
                pr = repp.tile([1, GF], F32, tag="reprelu")
                b2t = constp.tile([1, 1], F32)
                nc.gpsimd.memset(b2t[:], b2f)
                nc.scalar.activation(pr[:], psp[:], AF.Relu, bias=b2t[:])
                nc.sync.dma_start(preds_out[:], pr[:])


    nc.compile()
    return nc


_CACHE = {}


def _get_compiled(cfg, meta, sim_safe_gate):
    key = (cfg.n_nodes, cfg.n_edges, meta["TA"], meta["TB"], meta["b2f"],
           sim_safe_gate)
    if key not in _CACHE:
        _CACHE.clear()
        _CACHE[key] = _build(cfg, meta, sim_safe_gate=sim_safe_gate)
    return _CACHE[key]


def kernel(**inputs):
    cfg = REAL
    num_graphs = int(np.asarray(inputs["num_graphs"]))
    assert num_graphs <= cfg.graph_frame
    in_maps, meta = _prep(cfg, inputs)
    sim_safe_gate = os.environ.get("CGK_SAFE_GATE", "0") == "1"
    nc = _get_compiled(cfg, meta, sim_safe_gate)
    res = run_bass_kernel_spmd(nc, in_maps, core_ids=list(range(cfg.n_cores)))
    preds = np.asarray(res.results[0]["preds"], np.float32)
    return preds[0, :num_graphs].reshape(num_graphs, 1)


# exposed for test.py
def _run_prepared(nc, in_maps, n_cores):
    return run_bass_kernel_spmd(nc, in_maps, core_ids=list(range(n_cores)))



# revision 2
# speedup vs baseline: 385.3610x; 385.3610x over previous
"""CGConvNet (gnn_message_passing) Trainium2 kernel.

Strategy (8 NeuronCores, SPMD single program):
  - Nodes padded 50000 -> 50176 and sharded into 8 contiguous ranges of 6272
    (49 blocks of 128). Edges are assigned to the core owning their dst node.
  - Per core, edges are split into group A (src < 32768) and group B
    (src >= 32768) so gather indices fit in int16, then grouped by the
    128-node destination block with a uniform tiles-per-block padding so the
    program structure is identical on every core.
  - Per conv layer: each core computes its shard of the src-side table
    (v @ w_src) which is AllGathered into a full [50176, 64] DRAM table, and
    its local dst-side table (v @ w_dst).  Edge messages are assembled with
    dma_gather (256B rows), elementwise gate on ACT/DVE, and scatter-added
    into node blocks with one-hot matmuls accumulated in PSUM.
  - Graph pooling: one-hot matmul into a [64, 512] transposed graph frame,
    AllReduce across cores, then the tiny lin2 head.
"""

import math
import os
import sys

import numpy as np

sys.path.insert(0, "/opt/trn_rl_repo")

import concourse.bacc as bacc  # noqa: E402
import concourse.bass as bass  # noqa: E402
import concourse.mybir as mybir  # noqa: E402
import concourse.tile as tile  # noqa: E402
from concourse.bass_utils import run_bass_kernel_spmd  # noqa: E402
from concourse.library_config import mlp  # noqa: E402
from concourse.masks import make_identity  # noqa: E402

F32 = mybir.dt.float32
I16 = mybir.dt.int16
I32 = mybir.dt.int32
AF = mybir.ActivationFunctionType


class CFG:
    def __init__(self, n_nodes=50000, n_edges=800000, atom=100, bond=41,
                 hid=64, n_conv=3, num_graphs=500, n_cores=8, graph_frame=512):
        self.n_nodes, self.n_edges = n_nodes, n_edges
        self.atom, self.bond, self.hid = atom, bond, hid
        self.n_conv, self.num_graphs, self.n_cores = n_conv, num_graphs, n_cores
        # padded node count: multiple of 128 * n_cores
        q = 128 * n_cores
        self.npad = ((n_nodes + q - 1) // q) * q
        self.sh = self.npad // n_cores          # nodes per core
        self.nb = self.sh // 128                # 128-node blocks per core
        self.split = min(32768, (self.npad // 2 // 128) * 128)
        self.graph_frame = graph_frame
        assert num_graphs <= graph_frame
        assert self.npad - self.split <= 32768


REAL = CFG()


def _wrap_idx(a):
    """int16 index array -> [128, n/16] wrapped+replicated layout."""
    assert a.size % 16 == 0
    w = a.reshape(-1, 16).T  # [16, n/16]
    return np.tile(w, (8, 1)).copy()


def _prep(cfg, inputs):
    """Host-side data prep. Returns (in_maps, meta)."""
    src = np.asarray(inputs["src"]).astype(np.int64)
    dst = np.asarray(inputs["dst"]).astype(np.int64)
    gdf = np.asarray(inputs["gdf_feat"]).astype(np.float32)
    n_feat = np.asarray(inputs["n_feat"]).astype(np.float32)
    graph_ids = np.asarray(inputs["graph_ids"]).astype(np.int64)

    C, SH, NB, SPLIT = cfg.n_cores, cfg.sh, cfg.nb, cfg.split
    core_of = dst // SH

    # pass 1: per (core, group, block) counts -> uniform tile counts
    Tg = [0, 0]
    per = {}
    for c in range(C):
        in_core = core_of == c
        for g in range(2):
            m = in_core & ((src < SPLIT) if g == 0 else (src >= SPLIT))
            e = np.nonzero(m)[0]
            s = src[e]
            dloc = dst[e] - c * SH
            blk = dloc >> 7
            order = np.argsort(blk, kind="stable")
            e, s, dloc, blk = e[order], s[order], dloc[order], blk[order]
            counts = np.bincount(blk, minlength=NB)
            Tg[g] = max(Tg[g], int(math.ceil(counts.max() / 128.0)))
            per[(c, g)] = (e, s, dloc, blk, counts)
    TA, TB = max(Tg[0], 1), max(Tg[1], 1)

    # weights
    W0b = np.vstack([np.asarray(inputs["lin0_w"], np.float32),
                     np.asarray(inputs["lin0_b"], np.float32)[None]])
    conv_w = np.asarray(inputs["conv_w"], np.float32)
    H = cfg.hid
    Ws = np.ascontiguousarray(conv_w[:H])
    Wd = np.ascontiguousarray(conv_w[H:2 * H])
    Web = np.vstack([conv_w[2 * H:],
                     np.asarray(inputs["conv_b"], np.float32)[None]])
    w2 = np.asarray(inputs["lin2_w"], np.float32).reshape(H, 1)
    b2f = float(np.asarray(inputs["lin2_b"], np.float32).reshape(-1)[0])

    nf_pad = np.zeros((cfg.npad, cfg.atom), np.float32)
    nf_pad[:cfg.n_nodes] = n_feat
    gid_pad = np.full(cfg.npad, -1.0, np.float32)
    gid_pad[:cfg.n_nodes] = graph_ids.astype(np.float32)

    in_maps = []
    for c in range(C):
        m = {}
        # pass 2: padded per-group edge arrays
        for g, T in ((0, TA), (1, TB)):
            E = NB * T * 128
            e, s, dloc, blk, counts = per[(c, g)]
            starts = np.concatenate([[0], np.cumsum(counts)])
            within = np.arange(len(s)) - np.repeat(starts[:-1], counts)
            pos = blk * (T * 128) + within
            srcidx = np.zeros(E, np.int16)
            srcidx[pos] = (s if g == 0 else s - SPLIT).astype(np.int16)
            dstidx = np.zeros(E, np.int16)
            dstidx[pos] = dloc.astype(np.int16)
            dstrel = np.full(E, -1.0, np.float32)
            dstrel[pos] = (dloc - (blk << 7)).astype(np.float32)
            gdfT = np.zeros((cfg.bond + 1, E), np.float32)
            gdfT[:cfg.bond, pos] = gdf[e].T
            gdfT[cfg.bond, pos] = 1.0
            tag = "AB"[g]
            m[f"si{tag}"] = _wrap_idx(srcidx)
            m[f"di{tag}"] = _wrap_idx(dstidx)
            m[f"dr{tag}"] = np.ascontiguousarray(dstrel.reshape(-1, 128).T)
            m[f"gdfT{tag}"] = gdfT
        nfT = np.zeros((cfg.atom + 1, SH), np.float32)
        nfT[:cfg.atom] = nf_pad[c * SH:(c + 1) * SH].T
        nfT[cfg.atom] = 1.0
        m["nfT"] = nfT
        m["grel"] = np.ascontiguousarray(
            gid_pad[c * SH:(c + 1) * SH].reshape(NB, 128).T)
        m.update(W0b=W0b, Ws=Ws, Wd=Wd, Web=Web, w2=w2)
        in_maps.append(m)

    meta = dict(TA=TA, TB=TB, b2f=b2f)
    return in_maps, meta


def _build(cfg, meta, sim_safe_gate=True, debug_level=99):
    """Build the Bass program. Returns compiled nc."""
    C, SH, NB, SPLIT, H = cfg.n_cores, cfg.sh, cfg.nb, cfg.split, cfg.hid
    NPAD, GF = cfg.npad, cfg.graph_frame
    TA, TB, b2f = meta["TA"], meta["TB"], meta["b2f"]
    EA, EB = NB * TA * 128, NB * TB * 128
    AT1, BD1 = cfg.atom + 1, cfg.bond + 1
    CB = int(os.environ.get('CGK_CB', '2'))  # blocks per edge-chunk
    NQ = int(os.environ.get('CGK_NQ', '1'))

    def QN(q):
        return q % NQ

    nc = bacc.Bacc("TRN2", target_bir_lowering=False, debug=False,
                   num_devices=C, num_swdge_queues=4)

    din = {}
    for tag, T, E in (("A", TA, EA), ("B", TB, EB)):
        din[f"si{tag}"] = nc.dram_tensor(f"si{tag}", [128, E // 16], I16,
                                         kind="ExternalInput")
        din[f"di{tag}"] = nc.dram_tensor(f"di{tag}", [128, E // 16], I16,
                                         kind="ExternalInput")
        din[f"dr{tag}"] = nc.dram_tensor(f"dr{tag}", [128, E // 128], F32,
                                         kind="ExternalInput")
        din[f"gdfT{tag}"] = nc.dram_tensor(f"gdfT{tag}", [BD1, E], F32,
                                           kind="ExternalInput")
    din["nfT"] = nc.dram_tensor("nfT", [AT1, SH], F32, kind="ExternalInput")
    din["grel"] = nc.dram_tensor("grel", [128, NB], F32, kind="ExternalInput")
    din["W0b"] = nc.dram_tensor("W0b", [AT1, H], F32, kind="ExternalInput")
    din["Ws"] = nc.dram_tensor("Ws", [H, H], F32, kind="ExternalInput")
    din["Wd"] = nc.dram_tensor("Wd", [H, H], F32, kind="ExternalInput")
    din["Web"] = nc.dram_tensor("Web", [BD1, H], F32, kind="ExternalInput")
    din["w2"] = nc.dram_tensor("w2", [H, 1], F32, kind="ExternalInput")
    preds_out = nc.dram_tensor("preds", [1, GF], F32, kind="ExternalOutput")

    # chunk plan over blocks
    chunks = []
    b = 0
    while b < NB:
        n = min(CB, NB - b)
        chunks.append((b, n))
        b += n

    with tile.TileContext(nc) as tc:
        with tc.tile_pool(name="const", bufs=1) as constp, \
             tc.tile_pool(name="slab", bufs=1) as slabp, \
             tc.tile_pool(name="ph0", bufs=2) as ph0, \
             tc.tile_pool(name="work", bufs=2) as work, \
             tc.tile_pool(name="work3", bufs=3) as work3, \
             tc.tile_pool(name="small", bufs=2) as small, \
             tc.tile_pool(name="repp", bufs=1) as repp, \
             tc.tile_pool(name="psum", bufs=2, space="PSUM") as psum, \
             tc.tile_pool(name="psumt", bufs=1, space="PSUM") as psumt, \
             tc.tile_pool(name="psacc", bufs=2, space="PSUM") as psacc, \
             tc.tile_pool(name="psg", bufs=1, space="PSUM") as psgp, \
             tc.tile_pool(name="dram", bufs=1, space="DRAM") as dram:

            nc.gpsimd.load_library(mlp)

            # ---- constants ----
            w0b = constp.tile([AT1, H], F32)
            nc.sync.dma_start(w0b[:], din["W0b"][:])
            ws = constp.tile([H, H], F32)
            nc.sync.dma_start(ws[:], din["Ws"][:])
            wd = constp.tile([H, H], F32)
            nc.sync.dma_start(wd[:], din["Wd"][:])
            web = constp.tile([BD1, H], F32)
            nc.sync.dma_start(web[:], din["Web"][:])
            w2 = constp.tile([H, 1], F32)
            nc.sync.dma_start(w2[:], din["w2"][:])
            ident = constp.tile([128, 128], F32)
            make_identity(nc, ident[:])
            iota_i = constp.tile([128, 128], I32)
            nc.gpsimd.iota(iota_i[:], pattern=[[1, 128]], base=0,
                           channel_multiplier=0)
            iota_f = constp.tile([128, 128], F32)
            nc.vector.tensor_copy(iota_f[:], iota_i[:])
            iotg_i = constp.tile([128, GF], I32)
            nc.gpsimd.iota(iotg_i[:], pattern=[[1, GF]], base=0,
                           channel_multiplier=0)
            iotg_f = constp.tile([128, GF], F32)
            nc.vector.tensor_copy(iotg_f[:], iotg_i[:])
            grel = constp.tile([128, NB], F32)
            nc.sync.dma_start(grel[:], din["grel"][:])

            # index / dst_rel slabs (reused every layer)
            idx_sb = {}
            for tag, E in (("A", EA), ("B", EB)):
                for kind in ("si", "di"):
                    t = slabp.tile([128, E // 16], I16, tag=f"slab_{kind}{tag}")
                    nc.sync.dma_start(t[:], din[f"{kind}{tag}"][:])
                    idx_sb[f"{kind}{tag}"] = t
                t = slabp.tile([128, E // 128], F32, tag=f"slab_dr{tag}")
                nc.sync.dma_start(t[:], din[f"dr{tag}"][:])
                idx_sb[f"dr{tag}"] = t

            # persistent slabs
            accT = slabp.tile([H, SH], F32)          # transposed node feats
            acc3 = slabp.tile([128, NB, H], F32)     # final-layer node feats

            # internal DRAM
            egdf_dA = dram.tile([128, NB * TA, H], F32)
            egdf_dB = dram.tile([128, NB * TB, H], F32)
            egdf_d = {"A": egdf_dA, "B": egdf_dB}
            ts_full = dram.tile([NPAD, H], F32)      # AllGathered src table
            agS_in = dram.tile([SH, H], F32)
            vwD_d = dram.tile([SH, H], F32)
            rep_in = dram.tile([H, GF], F32)
            rep_out = dram.tile([H, GF], F32)

            # ---- phase 0a: egdf = gdfT.T @ Web (per group, once) ----
            for tag, T in (("A", TA), ("B", TB)):
                ntile_tot = NB * T
                step = 6
                for t0 in range(0, ntile_tot, step):
                    nt = min(step, ntile_tot - t0)
                    gsb = ph0.tile([BD1, nt * 128], F32, tag="gdfchunk")
                    nc.sync.dma_start(
                        gsb[:], din[f"gdfT{tag}"][:, t0 * 128:(t0 + nt) * 128])
                    esb = ph0.tile([128, nt, H], F32, tag="egdfchunk")
                    for t in range(nt):
                        ps = psum.tile([128, H], F32, tag="mm64")
                        nc.tensor.matmul(ps[:],
                                         lhsT=gsb[:, t * 128:(t + 1) * 128],
                                         rhs=web[:], start=True, stop=True)
                        nc.scalar.copy(esb[:, t, :], ps[:])
                    nc.sync.dma_start(
                        egdf_d[tag][:, t0:t0 + nt, :], esb[:])

            # ---- phase 0b: lin0 -> accT ----
            for blk in range(NB):
                nf_t = small.tile([AT1, 128], F32, tag="nf")
                nc.sync.dma_start(
                    nf_t[:], din["nfT"][:, blk * 128:(blk + 1) * 128])
                ps = psum.tile([128, H], F32, tag="mm64")
                nc.tensor.matmul(ps[:], lhsT=nf_t[:], rhs=w0b[:],
                                 start=True, stop=True)
                v0 = small.tile([128, H], F32, tag="v0")
                nc.scalar.activation(v0[:], ps[:], AF.Relu)
                pst = psumt.tile([H, 128], F32, tag="mmT")
                nc.tensor.transpose(pst[:], v0[:], ident[:])
                nc.scalar.copy(accT[:, blk * 128:(blk + 1) * 128], pst[:])

            _unused = None

            def _dbg_out(src_ap):
                pr = repp.tile([1, GF], F32, tag="reprelu")
                nc.scalar.copy(pr[:], src_ap)
                nc.sync.dma_start(preds_out[:], pr[:])

            if debug_level == 1:
                _dbg_out(accT[0:1, 0:GF])
            # ---- conv layers ----
            n_layers = 0 if debug_level == 1 else cfg.n_conv
            if debug_level == 2:
                n_layers = 1
            for layer in range(n_layers):
                last = layer == n_layers - 1
                # tables
                for blk in range(NB):
                    col = slice(blk * 128, (blk + 1) * 128)
                    ps = psum.tile([128, H], F32, tag="mm64")
                    nc.tensor.matmul(ps[:], lhsT=accT[:, col], rhs=ws[:],
                                     start=True, stop=True)
                    t_s = small.tile([128, H], F32, tag="tbl")
                    nc.scalar.copy(t_s[:], ps[:])
                    nc.sync.dma_start(agS_in[col, :], t_s[:])
                    ps2 = psum.tile([128, H], F32, tag="mm64")
                    nc.tensor.matmul(ps2[:], lhsT=accT[:, col], rhs=wd[:],
                                     start=True, stop=True)
                    t_d = small.tile([128, H], F32, tag="tbl")
                    nc.scalar.copy(t_d[:], ps2[:])
                    nc.sync.dma_start(vwD_d[col, :], t_d[:])
                if os.environ.get("CGK_NO_AG", "0") == "1":
                    nc.sync.dma_start(ts_full[:][0:SH, :], agS_in[:])
                else:
                    nc.gpsimd.collective_compute(
                        "AllGather", mybir.AluOpType.bypass,
                        replica_groups=[list(range(C))],
                        ins=[agS_in.opt()], outs=[ts_full.opt()])

                ts_lo = ts_full[:][0:SPLIT, :]
                ts_hi = ts_full[:][SPLIT:NPAD, :]

                sub = int(os.environ.get("CGK_SUB", "9"))
                for ci, (b0, nblk) in enumerate(chunks if sub >= 1 else []):
                    bufs = {}
                    for gi, (tag, T) in enumerate((("A", TA), ("B", TB))):
                        nt = nblk * T
                        t_lo = b0 * T
                        e_lo = t_lo * 128
                        # stable queue per gather stream: srcA=0 dstA=1
                        # srcB=2 dstB=3 (rotating queues trips SWDGE sem-lane
                        # locking in the scheduler model)
                        GMAX = int(os.environ.get("CGK_GMAX", "1024"))
                        zp = work3 if tag == "A" else work
                        z = zp.tile([128, nt, H], F32, tag=f"z{tag}")
                        for s0 in range(0, nt * 128, GMAX):
                            ni = min(GMAX, nt * 128 - s0)
                            nc.gpsimd.dma_gather(
                                z[:, s0 // 128:(s0 + ni) // 128, :],
                                ts_lo if tag == "A" else ts_hi,
                                idx_sb[f"si{tag}"][:, (e_lo + s0) // 16:(e_lo + s0 + ni) // 16],
                                ni, ni, H, queue_num=QN(2 * gi))
                        gd = work.tile([128, nt, H], F32, tag=f"gd{tag}")
                        for s0 in range(0, nt * 128, GMAX):
                            ni = min(GMAX, nt * 128 - s0)
                            nc.gpsimd.dma_gather(
                                gd[:, s0 // 128:(s0 + ni) // 128, :],
                                vwD_d[:],
                                idx_sb[f"di{tag}"][:, (e_lo + s0) // 16:(e_lo + s0 + ni) // 16],
                                ni, ni, H, queue_num=QN(2 * gi + 1))
                        eg = work.tile([128, nt, H], F32, tag=f"eg{tag}")
                        if os.environ.get("CGK_NO_EGL", "0") == "1":
                            nc.gpsimd.memset(eg[:], 0.1)
                        else:
                            nc.sync.dma_start(eg[:],
                                              egdf_d[tag][:, t_lo:t_lo + nt, :])
                        if sub < 3:
                            bufs[tag] = (z, None, T)
                            continue
                        # z = gS + gD + egdf
                        nc.vector.tensor_add(z[:], z[:], gd[:])
                        nc.vector.tensor_add(z[:], z[:], eg[:])
                        # gate: msg = sigmoid(z) * leaky_relu(z, 0.01)
                        sg = work.tile([128, nt, H], F32, tag=f"sg{tag}")
                        nc.scalar.activation(sg[:], z[:], AF.Sigmoid)
                        if sim_safe_gate:
                            # leaky = 0.505*z + 0.495*|z| (gd, eg reused)
                            nc.scalar.activation(gd[:], z[:], AF.Abs,
                                                 scale=0.495)
                            nc.scalar.mul(eg[:], z[:], 0.505)
                            lk = gd
                            nc.vector.tensor_add(lk[:], gd[:], eg[:])
                        else:
                            lk = gd
                            nc.scalar.activation(lk[:], z[:], AF.Lrelu,
                                                 alpha=0.01)
                        msg = z
                        nc.vector.tensor_mul(msg[:], sg[:], lk[:])
                        if sub < 4:
                            bufs[tag] = (msg, None, T)
                            continue
                        # one-hot vs dst_rel
                        oh = work.tile([128, nt, 128], F32, tag=f"oh{tag}")
                        dr = idx_sb[f"dr{tag}"][:, t_lo:t_lo + nt]
                        dr_b = bass.AP(dr.tensor, dr.offset,
                                       [dr.ap[0], dr.ap[1], [0, 128]])
                        io = iota_f[:]
                        io_b = bass.AP(io.tensor, io.offset,
                                       [io.ap[0], [0, nt], io.ap[1]])
                        nc.vector.tensor_tensor(out=oh[:], in0=dr_b, in1=io_b,
                                                op=mybir.AluOpType.is_equal)
                        bufs[tag] = (msg, oh, T)

                    for bi in range(nblk if sub >= 5 else 0):
                        blk = b0 + bi
                        seq = []
                        for tag in ("A", "B"):
                            msg, oh, T = bufs[tag]
                            for t in range(T):
                                seq.append((msg[:, bi * T + t, :],
                                            oh[:, bi * T + t, :]))
                        if not last:
                            ps = psacc.tile([H, 128], F32, tag="accps")
                            for k, (m_ap, o_ap) in enumerate(seq):
                                nc.tensor.matmul(ps[:], lhsT=m_ap, rhs=o_ap,
                                                 start=(k == 0),
                                                 stop=(k == len(seq) - 1))
                            nc.scalar.copy(
                                accT[:, blk * 128:(blk + 1) * 128], ps[:])
                        else:
                            ps = psacc.tile([128, H], F32, tag="accps")
                            for k, (m_ap, o_ap) in enumerate(seq):
                                nc.tensor.matmul(ps[:], lhsT=o_ap, rhs=m_ap,
                                                 start=(k == 0),
                                                 stop=(k == len(seq) - 1))
                            nc.scalar.copy(acc3[:, blk, :], ps[:])

            # ---- graph pooling ----
            if debug_level == 2:
                if int(os.environ.get("CGK_SUB", "9")) >= 5:
                    _dbg_out(acc3[0:1, :, :].rearrange("a b c -> a (b c)")[:, 0:GF])
                else:
                    _dbg_out(accT[0:1, 0:GF])
            do_pool = debug_level >= 3
            psg = None
            if do_pool:
                psg = psgp.tile([H, GF], F32, tag="repps")
            for blk in range(NB if do_pool else 0):
                ohg = small.tile([128, GF], F32, tag="ohg")
                gcol = grel[:, blk:blk + 1]
                nc.vector.tensor_tensor(
                    out=ohg[:], in0=gcol.to_broadcast([128, GF]),
                    in1=iotg_f[:], op=mybir.AluOpType.is_equal)
                nc.tensor.matmul(psg[:], lhsT=acc3[:, blk, :], rhs=ohg[:],
                                 start=(blk == 0), stop=(blk == NB - 1))
            if do_pool:
                rep_s = repp.tile([H, GF], F32, tag="reps")
                nc.scalar.copy(rep_s[:], psg[:])
                nc.sync.dma_start(rep_in[:], rep_s[:])
                if debug_level >= 4:
                    nc.gpsimd.collective_compute(
                        "AllReduce", mybir.AluOpType.add,
                        replica_groups=[list(range(C))],
                        ins=[rep_in.opt()], outs=[rep_out.opt()])
                else:
                    nc.sync.dma_start(rep_out[:], rep_s[:])
                rep_r = repp.tile([H, GF], F32, tag="reps")
                nc.sync.dma_start(rep_r[:], rep_out[:])
                rep_relu = repp.tile([H, GF], F32, tag="reprelu")
                nc.scalar.activation(rep_relu[:], rep_r[:], AF.Relu)
                psp = psgp.tile([1, GF], F32, tag="repps")
                nc.tensor.matmul(psp[:], lhsT=w2[:], rhs=rep_relu[:],
                                 start=True, stop=True)
                pr = repp.tile([1, GF], F32, tag="reprelu")
                b2t = constp.tile([1, 1], F32)
                nc.gpsimd.memset(b2t[:], b2f)
                nc.scalar.activation(pr[:], psp[:], AF.Relu, bias=b2t[:])
                nc.sync.dma_start(preds_out[:], pr[:])


    nc.compile()
    return nc


_CACHE = {}


def _get_compiled(cfg, meta, sim_safe_gate):
    key = (cfg.n_nodes, cfg.n_edges, meta["TA"], meta["TB"], meta["b2f"],
           sim_safe_gate)
    if key not in _CACHE:
        _CACHE.clear()
        _CACHE[key] = _build(cfg, meta, sim_safe_gate=sim_safe_gate)
    return _CACHE[key]


# ---------------------------------------------------------------------------
# Cached PJRT runner: run_bass_via_pjrt rebuilds its jit closure (and pays an
# XLA wrapper recompile) on every call; hoist it so repeat executions only pay
# transfer + dispatch + device execution.
# ---------------------------------------------------------------------------
_RUNNERS = {}


def _make_runner(nc, n_cores):
    key = id(nc)
    if key in _RUNNERS:
        return _RUNNERS[key]
    import jax
    from jax.sharding import Mesh, PartitionSpec, NamedSharding
    from jax.experimental.shard_map import shard_map
    from concourse import bass2jax

    bass2jax.install_neuronx_cc_hook()
    partition_name = (nc.partition_id_tensor.name
                      if nc.partition_id_tensor else None)
    in_names, out_names, out_avals = [], [], []
    for alloc in nc.m.functions[0].allocations:
        if not isinstance(alloc, mybir.MemoryLocationSet):
            continue
        name = alloc.memorylocations[0].name
        if alloc.kind == "ExternalInput":
            if name != partition_name:
                in_names.append(name)
        elif alloc.kind == "ExternalOutput":
            out_names.append(name)
            out_avals.append(jax.core.ShapedArray(
                tuple(alloc.tensor_shape), mybir.dt.np(alloc.dtype)))
    n_params, n_outs = len(in_names), len(out_avals)
    in_names_all = (in_names + out_names +
                    ([partition_name] if partition_name else []))

    def _body(*args):
        operands = list(args)
        if partition_name is not None:
            operands.append(bass2jax.partition_id_tensor())
        outs = bass2jax._bass_exec_p.bind(
            *operands, out_avals=tuple(out_avals),
            in_names=tuple(in_names_all), out_names=tuple(out_names),
            lowering_input_output_aliases=(),
            sim_require_finite=True, sim_require_nnan=True, nc=nc)
        return tuple(outs)

    devices = jax.devices()[:n_cores]
    mesh = Mesh(np.asarray(devices), ("core",))
    donate = tuple(range(n_params, n_params + n_outs))
    sharded = jax.jit(
        shard_map(_body, mesh=mesh,
                  in_specs=(PartitionSpec("core"),) * (n_params + n_outs),
                  out_specs=(PartitionSpec("core"),) * n_outs,
                  check_rep=False),
        donate_argnums=donate, keep_unused=True)

    class R:
        pass

    r = R()
    r.sharded = sharded
    r.in_names, r.out_names, r.out_avals = in_names, out_names, out_avals
    r.mesh = mesh
    r.shard = NamedSharding(mesh, PartitionSpec("core"))
    r.n_cores = n_cores
    _RUNNERS[key] = r
    return r


def _concat_inputs(r, in_maps):
    per_core = [[np.asarray(m[name]) for name in r.in_names] for m in in_maps]
    return [np.concatenate([per_core[c][i] for c in range(r.n_cores)], axis=0)
            for i in range(len(r.in_names))]


def _zero_outs(r):
    return [np.zeros((r.n_cores * a.shape[0], *a.shape[1:]), a.dtype)
            for a in r.out_avals]


def _execute(r, concat_in):
    """concat_in: list of np or device arrays matching r.in_names."""
    import jax
    outs = r.sharded(*concat_in, *_zero_outs(r))
    jax.block_until_ready(outs)
    return {name: np.asarray(outs[i]) for i, name in enumerate(r.out_names)}


def kernel(**inputs):
    cfg = REAL
    num_graphs = int(np.asarray(inputs["num_graphs"]))
    assert num_graphs <= cfg.graph_frame
    in_maps, meta = _prep(cfg, inputs)
    sim_safe_gate = os.environ.get("CGK_SAFE_GATE", "0") == "1"
    nc = _get_compiled(cfg, meta, sim_safe_gate)
    r = _make_runner(nc, cfg.n_cores)
    out = _execute(r, _concat_inputs(r, in_maps))
    preds = np.asarray(out["preds"], np.float32).reshape(cfg.n_cores, -1)
    return preds[0, :num_graphs].reshape(num_graphs, 1)



# revision 4
# speedup vs baseline: 1821.6229x; 4.7271x over previous
"""CGConvNet (gnn_message_passing) Trainium2 kernel.

Strategy (8 NeuronCores, SPMD single program):
  - Nodes padded 50000 -> 50176 and sharded into 8 contiguous ranges of 6272
    (49 blocks of 128). Edges are assigned to the core owning their dst node.
  - Per core, edges are split into group A (src < 32768) and group B
    (src >= 32768) so gather indices fit in int16, then grouped by the
    128-node destination block with a uniform tiles-per-block padding so the
    program structure is identical on every core.
  - Per conv layer: each core computes its shard of the src-side table
    (v @ w_src) which is AllGathered into a full [50176, 64] DRAM table, and
    its local dst-side table (v @ w_dst).  Edge messages are assembled with
    dma_gather (256B rows, 4 SWDGE queues), the bond-feature term is computed
    per tile on the PE (gdfT bf16 @ Web) directly into PSUM (no DRAM edge
    table), elementwise gate on ACT/DVE, and scatter-add into node blocks
    with one-hot matmuls (bf16) accumulated in PSUM.
  - Graph pooling: one-hot matmul into a [64, 512] transposed graph frame,
    AllReduce across cores, then the tiny lin2 head.
"""

import math
import os
import sys

import numpy as np

sys.path.insert(0, "/opt/trn_rl_repo")

import concourse.bacc as bacc  # noqa: E402
import concourse.bass as bass  # noqa: E402
import concourse.mybir as mybir  # noqa: E402
import concourse.tile as tile  # noqa: E402
from concourse.library_config import mlp  # noqa: E402

F32 = mybir.dt.float32
BF16 = mybir.dt.bfloat16
I16 = mybir.dt.int16
I32 = mybir.dt.int32
AF = mybir.ActivationFunctionType
NP_BF16 = mybir.dt.np(BF16)


class CFG:
    def __init__(self, n_nodes=50000, n_edges=800000, atom=100, bond=41,
                 hid=64, n_conv=3, num_graphs=500, n_cores=8, graph_frame=512):
        self.n_nodes, self.n_edges = n_nodes, n_edges
        self.atom, self.bond, self.hid = atom, bond, hid
        self.n_conv, self.num_graphs, self.n_cores = n_conv, num_graphs, n_cores
        # padded node count: multiple of 128 * n_cores
        q = 128 * n_cores
        self.npad = ((n_nodes + q - 1) // q) * q
        self.sh = self.npad // n_cores          # nodes per core
        self.nb = self.sh // 128                # 128-node blocks per core
        self.split = min(32768, (self.npad // 2 // 128) * 128)
        self.graph_frame = graph_frame
        assert num_graphs <= graph_frame
        assert self.npad - self.split <= 32768


REAL = CFG()


def _wrap16(a):
    """int16 index array -> [16, n/16] wrapped layout (device replicates)."""
    assert a.size % 16 == 0
    return np.ascontiguousarray(a.reshape(-1, 16).T)


def _prep(cfg, inputs):
    """Host-side data prep. Returns (in_maps, meta)."""
    src = np.asarray(inputs["src"]).astype(np.int64)
    dst = np.asarray(inputs["dst"]).astype(np.int64)
    gdf = np.asarray(inputs["gdf_feat"]).astype(np.float32)
    n_feat = np.asarray(inputs["n_feat"]).astype(np.float32)
    graph_ids = np.asarray(inputs["graph_ids"]).astype(np.int64)

    C, SH, NB, SPLIT = cfg.n_cores, cfg.sh, cfg.nb, cfg.split
    core_of = dst // SH

    # pass 1: per (core, group, block) counts -> uniform tile counts
    Tg = [0, 0]
    per = {}
    for c in range(C):
        in_core = core_of == c
        for g in range(2):
            m = in_core & ((src < SPLIT) if g == 0 else (src >= SPLIT))
            e = np.nonzero(m)[0]
            s = src[e]
            dloc = dst[e] - c * SH
            blk = dloc >> 7
            order = np.argsort(blk, kind="stable")
            e, s, dloc, blk = e[order], s[order], dloc[order], blk[order]
            counts = np.bincount(blk, minlength=NB)
            Tg[g] = max(Tg[g], int(math.ceil(counts.max() / 128.0)))
            per[(c, g)] = (e, s, dloc, blk, counts)
    TA, TB = max(Tg[0], 1), max(Tg[1], 1)

    # weights
    W0b = np.vstack([np.asarray(inputs["lin0_w"], np.float32),
                     np.asarray(inputs["lin0_b"], np.float32)[None]])
    conv_w = np.asarray(inputs["conv_w"], np.float32)
    H = cfg.hid
    Ws = np.ascontiguousarray(conv_w[:H])
    Wd = np.ascontiguousarray(conv_w[H:2 * H])
    Web = np.vstack([conv_w[2 * H:],
                     np.asarray(inputs["conv_b"], np.float32)[None]])
    w2 = np.asarray(inputs["lin2_w"], np.float32).reshape(H, 1)
    b2f = float(np.asarray(inputs["lin2_b"], np.float32).reshape(-1)[0])

    nf_pad = np.zeros((cfg.npad, cfg.atom), np.float32)
    nf_pad[:cfg.n_nodes] = n_feat
    gid_pad = np.full(cfg.npad, -1.0, np.float32)
    gid_pad[:cfg.n_nodes] = graph_ids.astype(np.float32)

    in_maps = []
    for c in range(C):
        m = {}
        # pass 2: padded per-group edge arrays
        for g, T in ((0, TA), (1, TB)):
            E = NB * T * 128
            e, s, dloc, blk, counts = per[(c, g)]
            starts = np.concatenate([[0], np.cumsum(counts)])
            within = np.arange(len(s)) - np.repeat(starts[:-1], counts)
            pos = blk * (T * 128) + within
            srcidx = np.zeros(E, np.int16)
            srcidx[pos] = (s if g == 0 else s - SPLIT).astype(np.int16)
            dstidx = np.zeros(E, np.int16)
            dstidx[pos] = dloc.astype(np.int16)
            dstrel = np.full(E, -1.0, np.float32)
            dstrel[pos] = (dloc - (blk << 7)).astype(np.float32)
            gdfT = np.zeros((cfg.bond + 1, E), NP_BF16)
            gdfT[:cfg.bond, pos] = gdf[e].astype(NP_BF16).T
            gdfT[cfg.bond, pos] = 1.0
            tag = "AB"[g]
            m[f"si{tag}"] = _wrap16(srcidx)
            m[f"di{tag}"] = _wrap16(dstidx)
            m[f"dr{tag}"] = np.ascontiguousarray(dstrel.reshape(-1, 128).T)
            m[f"gdfT{tag}"] = gdfT
        nfT = np.zeros((cfg.atom + 1, SH), NP_BF16)
        nfT[:cfg.atom] = nf_pad[c * SH:(c + 1) * SH].astype(NP_BF16).T
        nfT[cfg.atom] = 1.0
        m["nfT"] = nfT
        m["grel"] = np.ascontiguousarray(
            gid_pad[c * SH:(c + 1) * SH].reshape(NB, 128).T)
        m.update(W0b=W0b.astype(NP_BF16), Ws=Ws, Wd=Wd,
                 Web=Web.astype(NP_BF16), w2=w2)
        in_maps.append(m)

    meta = dict(TA=TA, TB=TB, b2f=b2f)
    return in_maps, meta


def _build(cfg, meta, sim_safe_gate=True):
    """Build the Bass program. Returns compiled nc."""
    C, SH, NB, SPLIT, H = cfg.n_cores, cfg.sh, cfg.nb, cfg.split, cfg.hid
    NPAD, GF = cfg.npad, cfg.graph_frame
    TA, TB, b2f = meta["TA"], meta["TB"], meta["b2f"]
    EA, EB = NB * TA * 128, NB * TB * 128
    AT1, BD1 = cfg.atom + 1, cfg.bond + 1
    NQ = int(os.environ.get('CGK_NQ', '4'))
    GMAX = int(os.environ.get("CGK_GMAX", "1024"))

    nc = bacc.Bacc("TRN2", target_bir_lowering=False, debug=False,
                   num_devices=C, num_swdge_queues=4)

    din = {}
    for tag, T, E in (("A", TA, EA), ("B", TB, EB)):
        din[f"si{tag}"] = nc.dram_tensor(f"si{tag}", [16, E // 16], I16,
                                         kind="ExternalInput")
        din[f"di{tag}"] = nc.dram_tensor(f"di{tag}", [16, E // 16], I16,
                                         kind="ExternalInput")
        din[f"dr{tag}"] = nc.dram_tensor(f"dr{tag}", [128, E // 128], F32,
                                         kind="ExternalInput")
        din[f"gdfT{tag}"] = nc.dram_tensor(f"gdfT{tag}", [BD1, E], BF16,
                                           kind="ExternalInput")
    din["nfT"] = nc.dram_tensor("nfT", [AT1, SH], BF16, kind="ExternalInput")
    din["grel"] = nc.dram_tensor("grel", [128, NB], F32, kind="ExternalInput")
    din["W0b"] = nc.dram_tensor("W0b", [AT1, H], BF16, kind="ExternalInput")
    din["Ws"] = nc.dram_tensor("Ws", [H, H], F32, kind="ExternalInput")
    din["Wd"] = nc.dram_tensor("Wd", [H, H], F32, kind="ExternalInput")
    din["Web"] = nc.dram_tensor("Web", [BD1, H], BF16, kind="ExternalInput")
    din["w2"] = nc.dram_tensor("w2", [H, 1], F32, kind="ExternalInput")
    preds_out = nc.dram_tensor("preds", [1, GF], F32, kind="ExternalOutput")

    with tile.TileContext(nc) as tc:
        with tc.tile_pool(name="const", bufs=1) as constp, \
             tc.tile_pool(name="slab", bufs=1) as slabp, \
             tc.tile_pool(name="work", bufs=2) as work, \
             tc.tile_pool(name="small", bufs=2) as small, \
             tc.tile_pool(name="repp", bufs=1) as repp, \
             tc.tile_pool(name="pmm", bufs=1, space="PSUM") as pmm, \
             tc.tile_pool(name="pacc", bufs=1, space="PSUM") as pacc, \
             tc.tile_pool(name="ppsz", bufs=1, space="PSUM") as ppsz, \
             tc.tile_pool(name="ppsg", bufs=1, space="PSUM") as ppsg, \
             tc.tile_pool(name="dram", bufs=1, space="DRAM") as dram:

            nc.gpsimd.load_library(mlp)

            # ---- constants ----
            w0b = constp.tile([AT1, H], BF16)
            nc.sync.dma_start(w0b[:], din["W0b"][:])
            ws = constp.tile([H, H], F32)
            nc.sync.dma_start(ws[:], din["Ws"][:])
            wd = constp.tile([H, H], F32)
            nc.sync.dma_start(wd[:], din["Wd"][:])
            web = constp.tile([BD1, H], BF16)
            nc.sync.dma_start(web[:], din["Web"][:])
            w2 = constp.tile([H, 1], F32)
            nc.sync.dma_start(w2[:], din["w2"][:])
            iota_i = constp.tile([128, 128], I32)
            nc.gpsimd.iota(iota_i[:], pattern=[[1, 128]], base=0,
                           channel_multiplier=0)
            iota_f = constp.tile([128, 128], F32)
            nc.vector.tensor_copy(iota_f[:], iota_i[:])
            iotg_i = constp.tile([128, GF], I32)
            nc.gpsimd.iota(iotg_i[:], pattern=[[1, GF]], base=0,
                           channel_multiplier=0)
            iotg_f = constp.tile([128, GF], F32)
            nc.vector.tensor_copy(iotg_f[:], iotg_i[:])
            grel = constp.tile([128, NB], F32)
            nc.sync.dma_start(grel[:], din["grel"][:])

            # index / dst_rel slabs (reused every layer); si/di arrive
            # de-replicated [16, E/16] and are replicated into 128
            # partitions on-device (dma_gather requires that layout).
            idx_sb = {}
            for tag, E in (("A", EA), ("B", EB)):
                for kind in ("si", "di"):
                    t = slabp.tile([128, E // 16], I16, tag=f"slab_{kind}{tag}")
                    for k in range(8):
                        nc.sync.dma_start(t[16 * k:16 * (k + 1), :],
                                          din[f"{kind}{tag}"][:])
                    idx_sb[f"{kind}{tag}"] = t
                t = slabp.tile([128, E // 128], F32, tag=f"slab_dr{tag}")
                nc.sync.dma_start(t[:], din[f"dr{tag}"][:])
                idx_sb[f"dr{tag}"] = t

            # persistent slabs
            accT = slabp.tile([H, SH], F32)          # transposed node feats
            acc3 = slabp.tile([128, NB, H], F32)     # final-layer node feats

            # internal DRAM
            ts_full = dram.tile([NPAD, H], F32)      # AllGathered src table
            agS_in = dram.tile([SH, H], F32)
            vwD_d = dram.tile([SH, H], F32)
            rep_in = dram.tile([H, GF], F32)
            rep_out = dram.tile([H, GF], F32)

            # ---- phase 0: lin0 -> accT (no transpose: out = W0b.T @ nfT) ----
            for blk in range(NB):
                nf_t = small.tile([AT1, 128], BF16, tag="nf")
                nc.sync.dma_start(
                    nf_t[:], din["nfT"][:, blk * 128:(blk + 1) * 128])
                ps = pmm.tile([H, 128], F32, tag="mmA")
                nc.tensor.matmul(ps[:], lhsT=w0b[:], rhs=nf_t[:],
                                 start=True, stop=True)
                nc.scalar.activation(
                    accT[:, blk * 128:(blk + 1) * 128], ps[:], AF.Relu)

            # ---- conv layers ----
            for layer in range(cfg.n_conv):
                last = layer == cfg.n_conv - 1
                # tables: ts (to be AllGathered) and local vd
                for blk in range(NB):
                    col = slice(blk * 128, (blk + 1) * 128)
                    ps = pmm.tile([128, H], F32, tag="mm64")
                    nc.tensor.matmul(ps[:], lhsT=accT[:, col], rhs=ws[:],
                                     start=True, stop=True)
                    t_s = small.tile([128, H], F32, tag="tbl")
                    nc.scalar.copy(t_s[:], ps[:])
                    nc.sync.dma_start(agS_in[col, :], t_s[:])
                    ps2 = pmm.tile([128, H], F32, tag="mm64")
                    nc.tensor.matmul(ps2[:], lhsT=accT[:, col], rhs=wd[:],
                                     start=True, stop=True)
                    t_d = small.tile([128, H], F32, tag="tbl")
                    nc.scalar.copy(t_d[:], ps2[:])
                    nc.sync.dma_start(vwD_d[col, :], t_d[:])
                nc.gpsimd.collective_compute(
                    "AllGather", mybir.AluOpType.bypass,
                    replica_groups=[list(range(C))],
                    ins=[agS_in.opt()], outs=[ts_full.opt()])

                ts_lo = ts_full[:][0:SPLIT, :]
                ts_hi = ts_full[:][SPLIT:NPAD, :]

                for blk in range(NB):
                    bufs = {}
                    for gi, (tag, T) in enumerate((("A", TA), ("B", TB))):
                        nt = T
                        t_lo = blk * T
                        e_lo = t_lo * 128
                        zsrc = work.tile([128, nt, H], F32, tag=f"zs{tag}")
                        for s0 in range(0, nt * 128, GMAX):
                            ni = min(GMAX, nt * 128 - s0)
                            nc.gpsimd.dma_gather(
                                zsrc[:, s0 // 128:(s0 + ni) // 128, :],
                                ts_lo if tag == "A" else ts_hi,
                                idx_sb[f"si{tag}"][:, (e_lo + s0) // 16:(e_lo + s0 + ni) // 16],
                                ni, ni, H, queue_num=(2 * gi) % NQ)
                        zdst = work.tile([128, nt, H], F32, tag=f"zd{tag}")
                        for s0 in range(0, nt * 128, GMAX):
                            ni = min(GMAX, nt * 128 - s0)
                            nc.gpsimd.dma_gather(
                                zdst[:, s0 // 128:(s0 + ni) // 128, :],
                                vwD_d[:],
                                idx_sb[f"di{tag}"][:, (e_lo + s0) // 16:(e_lo + s0 + ni) // 16],
                                ni, ni, H, queue_num=(2 * gi + 1) % NQ)
                        # bond-feature term on the PE, straight into PSUM
                        gsb = work.tile([BD1, nt * 128], BF16, tag=f"g{tag}")
                        nc.sync.dma_start(
                            gsb[:],
                            din[f"gdfT{tag}"][:, e_lo:e_lo + nt * 128])
                        psz = ppsz.tile([128, nt, H], F32, tag="psz")
                        for t in range(nt):
                            nc.tensor.matmul(
                                psz[:, t, :],
                                lhsT=gsb[:, t * 128:(t + 1) * 128],
                                rhs=web[:], start=True, stop=True)
                        # z = egdf + gS + gD
                        z = work.tile([128, nt, H], F32, tag=f"z{tag}")
                        nc.vector.tensor_add(z[:], psz[:], zsrc[:])
                        nc.vector.tensor_add(z[:], z[:], zdst[:])
                        # gate: msg = sigmoid(z) * leaky_relu(z, 0.01)
                        sg = work.tile([128, nt, H], F32, tag=f"sg{tag}")
                        nc.scalar.activation(sg[:], z[:], AF.Sigmoid)
                        lk = work.tile([128, nt, H], F32, tag=f"lk{tag}")
                        if sim_safe_gate:
                            # leaky = 0.505*z + 0.495*|z|
                            nc.scalar.activation(lk[:], z[:], AF.Abs,
                                                 scale=0.495)
                            tmp = work.tile([128, nt, H], F32, tag=f"tm{tag}")
                            nc.scalar.mul(tmp[:], z[:], 0.505)
                            nc.vector.tensor_add(lk[:], lk[:], tmp[:])
                        else:
                            nc.scalar.activation(lk[:], z[:], AF.Lrelu,
                                                 alpha=0.01)
                        msg = work.tile([128, nt, H], BF16, tag=f"ms{tag}")
                        nc.vector.tensor_mul(msg[:], sg[:], lk[:])
                        # one-hot vs dst_rel
                        oh = work.tile([128, nt, 128], BF16, tag=f"oh{tag}")
                        dr = idx_sb[f"dr{tag}"][:, t_lo:t_lo + nt]
                        dr_b = bass.AP(dr.tensor, dr.offset,
                                       [dr.ap[0], dr.ap[1], [0, 128]])
                        io = iota_f[:]
                        io_b = bass.AP(io.tensor, io.offset,
                                       [io.ap[0], [0, nt], io.ap[1]])
                        nc.vector.tensor_tensor(out=oh[:], in0=dr_b, in1=io_b,
                                                op=mybir.AluOpType.is_equal)
                        bufs[tag] = (msg, oh, T)

                    seq = []
                    for tag in ("A", "B"):
                        msg, oh, T = bufs[tag]
                        for t in range(T):
                            seq.append((msg[:, t, :], oh[:, t, :]))
                    if not last:
                        ps = pacc.tile([H, 128], F32, tag="accps")
                        for k, (m_ap, o_ap) in enumerate(seq):
                            nc.tensor.matmul(ps[:], lhsT=m_ap, rhs=o_ap,
                                             start=(k == 0),
                                             stop=(k == len(seq) - 1))
                        nc.scalar.copy(
                            accT[:, blk * 128:(blk + 1) * 128], ps[:])
                    else:
                        ps = pacc.tile([128, H], F32, tag="accL")
                        for k, (m_ap, o_ap) in enumerate(seq):
                            nc.tensor.matmul(ps[:], lhsT=o_ap, rhs=m_ap,
                                             start=(k == 0),
                                             stop=(k == len(seq) - 1))
                        nc.scalar.copy(acc3[:, blk, :], ps[:])

            # ---- graph pooling ----
            psg = ppsg.tile([H, GF], F32, tag="repps")
            for blk in range(NB):
                ohg = small.tile([128, GF], F32, tag="ohg")
                gcol = grel[:, blk:blk + 1]
                nc.vector.tensor_tensor(
                    out=ohg[:], in0=gcol.to_broadcast([128, GF]),
                    in1=iotg_f[:], op=mybir.AluOpType.is_equal)
                nc.tensor.matmul(psg[:], lhsT=acc3[:, blk, :], rhs=ohg[:],
                                 start=(blk == 0), stop=(blk == NB - 1))
            rep_s = repp.tile([H, GF], F32, tag="reps")
            nc.scalar.copy(rep_s[:], psg[:])
            nc.sync.dma_start(rep_in[:], rep_s[:])
            nc.gpsimd.collective_compute(
                "AllReduce", mybir.AluOpType.add,
                replica_groups=[list(range(C))],
                ins=[rep_in.opt()], outs=[rep_out.opt()])
            rep_r = repp.tile([H, GF], F32, tag="reps")
            nc.sync.dma_start(rep_r[:], rep_out[:])
            rep_relu = repp.tile([H, GF], F32, tag="reprelu")
            nc.scalar.activation(rep_relu[:], rep_r[:], AF.Relu)
            psp = ppsg.tile([1, GF], F32, tag="psp")
            nc.tensor.matmul(psp[:], lhsT=w2[:], rhs=rep_relu[:],
                             start=True, stop=True)
            pr = repp.tile([1, GF], F32, tag="reprelu")
            b2t = constp.tile([1, 1], F32)
            nc.gpsimd.memset(b2t[:], b2f)
            nc.scalar.activation(pr[:], psp[:], AF.Relu, bias=b2t[:])
            nc.sync.dma_start(preds_out[:], pr[:])

    nc.compile()
    return nc


_CACHE = {}


def _get_compiled(cfg, meta, sim_safe_gate):
    key = (cfg.n_nodes, cfg.n_edges, meta["TA"], meta["TB"], meta["b2f"],
           sim_safe_gate)
    if key not in _CACHE:
        _CACHE.clear()
        _CACHE[key] = _build(cfg, meta, sim_safe_gate=sim_safe_gate)
    return _CACHE[key]


# ---------------------------------------------------------------------------
# Cached PJRT runner: run_bass_via_pjrt rebuilds its jit closure (and pays an
# XLA wrapper recompile) on every call; hoist it so repeat executions only pay
# transfer + dispatch + device execution.
# ---------------------------------------------------------------------------
_RUNNERS = {}


def _make_runner(nc, n_cores):
    key = id(nc)
    if key in _RUNNERS:
        return _RUNNERS[key]
    import jax
    from jax.sharding import Mesh, PartitionSpec, NamedSharding
    from jax.experimental.shard_map import shard_map
    from concourse import bass2jax

    bass2jax.install_neuronx_cc_hook()
    partition_name = (nc.partition_id_tensor.name
                      if nc.partition_id_tensor else None)
    in_names, out_names, out_avals = [], [], []
    for alloc in nc.m.functions[0].allocations:
        if not isinstance(alloc, mybir.MemoryLocationSet):
            continue
        name = alloc.memorylocations[0].name
        if alloc.kind == "ExternalInput":
            if name != partition_name:
                in_names.append(name)
        elif alloc.kind == "ExternalOutput":
            out_names.append(name)
            out_avals.append(jax.core.ShapedArray(
                tuple(alloc.tensor_shape), mybir.dt.np(alloc.dtype)))
    n_params, n_outs = len(in_names), len(out_avals)
    in_names_all = (in_names + out_names +
                    ([partition_name] if partition_name else []))

    def _body(*args):
        operands = list(args)
        if partition_name is not None:
            operands.append(bass2jax.partition_id_tensor())
        outs = bass2jax._bass_exec_p.bind(
            *operands, out_avals=tuple(out_avals),
            in_names=tuple(in_names_all), out_names=tuple(out_names),
            lowering_input_output_aliases=(),
            sim_require_finite=True, sim_require_nnan=True, nc=nc)
        return tuple(outs)

    devices = jax.devices()[:n_cores]
    mesh = Mesh(np.asarray(devices), ("core",))
    donate = tuple(range(n_params, n_params + n_outs))
    sharded = jax.jit(
        shard_map(_body, mesh=mesh,
                  in_specs=(PartitionSpec("core"),) * (n_params + n_outs),
                  out_specs=(PartitionSpec("core"),) * n_outs,
                  check_rep=False),
        donate_argnums=donate, keep_unused=True)

    class R:
        pass

    r = R()
    r.sharded = sharded
    r.in_names, r.out_names, r.out_avals = in_names, out_names, out_avals
    r.mesh = mesh
    r.shard = NamedSharding(mesh, PartitionSpec("core"))
    r.n_cores = n_cores
    _RUNNERS[key] = r
    return r


def _concat_inputs(r, in_maps):
    per_core = [[np.asarray(m[name]) for name in r.in_names] for m in in_maps]
    return [np.concatenate([per_core[c][i] for c in range(r.n_cores)], axis=0)
            for i in range(len(r.in_names))]


def _zero_outs(r):
    return [np.zeros((r.n_cores * a.shape[0], *a.shape[1:]), a.dtype)
            for a in r.out_avals]


def _execute(r, concat_in):
    """concat_in: list of np or device arrays matching r.in_names."""
    import jax
    outs = r.sharded(*concat_in, *_zero_outs(r))
    jax.block_until_ready(outs)
    return {name: np.asarray(outs[i]) for i, name in enumerate(r.out_names)}


def kernel(**inputs):
    cfg = REAL
    num_graphs = int(np.asarray(inputs["num_graphs"]))
    assert num_graphs <= cfg.graph_frame
    in_maps, meta = _prep(cfg, inputs)
    sim_safe_gate = os.environ.get("CGK_SAFE_GATE", "0") == "1"
    nc = _get_compiled(cfg, meta, sim_safe_gate)
    r = _make_runner(nc, cfg.n_cores)
    out = _execute(r, _concat_inputs(r, in_maps))
    preds = np.asarray(out["preds"], np.float32).reshape(cfg.n_cores, -1)
    return preds[0, :num_graphs].reshape(num_graphs, 1)
